# revision 4
# baseline (speedup 1.0000x reference)
"""v2: DVE-staged contiguous stores (32KB descriptors) + raw-bass DMA pipeline.

Per tensor (x on SP ring, y on ACT ring):
  - 2 load DMAs (b=0, b=1) into a column+row padded SBUF tile.
  - DVE copies each patch window [64, ROWS*W] into a contiguous stage
    sub-slot; stores then read contiguous SBUF -> one 32KB descriptor per
    partition-channel instead of 32x 1KB.
  - Stage pool per tensor: NSTAGE tiles [128, ROWS*W]; b=0 patches use
    partitions 0-63 of a tile, b=1 patches use 64-127 (DVE is
    partition-preserving), giving NSTAGE independent sub-slots per b.
Pipeline: copy m -> store m; copy m waits for store m-NSTAGE (sub-slot reuse).
"""

import os
import sys

import numpy as np

try:
    import concourse  # noqa: F401
except ImportError:
    for p in ("/root/.axon_site", "/root/.axon_site/_ro/trn_rl_repo",
              "/root/.axon_site/_ro/pypackages", "/opt/trn_rl_repo"):
        if os.path.isdir(p) and p not in sys.path:
            sys.path.append(p)

import concourse.bass as bass
import concourse.mybir as mybir
from concourse.bass_utils import run_bass_kernel_spmd

N_CORES = 8
B, C, H, W = 2, 64, 256, 256
F = 3
ROWS = H // N_CORES  # 32
NSTAGE = 2  # stage tiles per tensor (sub-slot depth per b)

_cache = {}


def _build_nc(d: int) -> bass.Bass:
    PR = ROWS + 2 * d
    PW = W + 2 * d
    PATCH = ROWS * W  # 8192 elements per channel per patch
    f32 = mybir.dt.float32

    # pure-HWDGE kernel: shrink the (unused) SWDGE descriptor-ring carveout
    # so the stage tiles fit in SBUF.
    nc = bass.Bass("TRN2", dynamic_dma_scratch_size=2048)
    xs = nc.dram_tensor("xs", [B * C, PR, PW], f32, kind="ExternalInput")
    ys = nc.dram_tensor("ys", [B * C, PR, PW], f32, kind="ExternalInput")
    ox = nc.dram_tensor("ox", [B, F * F * C, PATCH], f32, kind="ExternalOutput")
    oy = nc.dram_tensor("oy", [B, F * F * C, PATCH], f32, kind="ExternalOutput")

    from contextlib import ExitStack

    with ExitStack() as ctx:
        tx = ctx.enter_context(nc.sbuf_tensor("tx", [B * C, PR, PW], f32))
        ty = ctx.enter_context(nc.sbuf_tensor("ty", [B * C, PR, PW], f32))
        stx = [
            ctx.enter_context(nc.sbuf_tensor(f"stx{i}", [B * C, PATCH], f32))
            for i in range(NSTAGE)
        ]
        sty = [
            ctx.enter_context(nc.sbuf_tensor(f"sty{i}", [B * C, PATCH], f32))
            for i in range(NSTAGE)
        ]
        xl_sem = ctx.enter_context(nc.semaphore("xl"))
        yl_sem = ctx.enter_context(nc.semaphore("yl"))
        xc_sem = ctx.enter_context(nc.semaphore("xc"))
        yc_sem = ctx.enter_context(nc.semaphore("yc"))
        xs_sem = ctx.enter_context(nc.semaphore("xst"))
        ys_sem = ctx.enter_context(nc.semaphore("yst"))
        block = ctx.enter_context(nc.Block())

        # copy/store order per tensor: m = b*9 + k  (all b=0 first)
        def windows(m):
            b, k = divmod(m, F * F)
            i, j = divmod(k, F)
            return b, k, i, j

        def emit_dma(eng, src, dst, tile, stage, load_sem, copy_sem, store_sem):
            # loads: b=0 then b=1
            for b in range(B):
                eng.dma_start(
                    out=tile[b * C : (b + 1) * C],
                    in_=src[b * C : (b + 1) * C],
                ).then_inc(load_sem, 16)
            for m in range(B * F * F):
                b, k, i, j = windows(m)
                slot = stage[m % NSTAGE]
                eng.wait_ge(copy_sem, m + 1)
                eng.dma_start(
                    out=dst[b, k * C : (k + 1) * C, :],
                    in_=slot[b * C : (b + 1) * C],
                ).then_inc(store_sem, 16)
            eng.wait_ge(store_sem, 16 * B * F * F)

        def emit_copy(vector, which):
            # interleave x and y patch copies
            for m in range(B * F * F):
                for tile, stage, load_sem, copy_sem, store_sem in which:
                    b, k, i, j = windows(m)
                    slot = stage[m % NSTAGE]
                    vector.wait_ge(load_sem, 16 * (b + 1))
                    if m >= NSTAGE:
                        vector.wait_ge(store_sem, 16 * (m - NSTAGE + 1))
                    vector.tensor_copy(
                        out=slot[b * C : (b + 1) * C].rearrange(
                            "c (r w) -> c r w", r=ROWS
                        ),
                        in_=tile[
                            b * C : (b + 1) * C,
                            i * d : i * d + ROWS,
                            j * d : j * d + W,
                        ],
                    ).then_inc(copy_sem)

        @block.sync
        def _(sync):
            emit_dma(sync, xs, ox, tx, stx, xl_sem, xc_sem, xs_sem)

        @block.scalar
        def _(scalar):
            emit_dma(scalar, ys, oy, ty, sty, yl_sem, yc_sem, ys_sem)

        @block.vector
        def _(vector):
            emit_copy(
                vector,
                [
                    (tx, stx, xl_sem, xc_sem, xs_sem),
                    (ty, sty, yl_sem, yc_sem, ys_sem),
                ],
            )

    return nc


def kernel(inref_x: np.ndarray, inref_y: np.ndarray, dilation) -> tuple:
    d = int(dilation)
    x = np.ascontiguousarray(np.asarray(inref_x, dtype=np.float32))
    y = np.ascontiguousarray(np.asarray(inref_y, dtype=np.float32))

    if d not in _cache:
        _cache[d] = _build_nc(d)
    nc = _cache[d]

    px = np.pad(x, ((0, 0), (0, 0), (d, d), (d, d)), mode="reflect")
    py = np.pad(y, ((0, 0), (0, 0), (d, d), (d, d)), mode="reflect")
    PR = ROWS + 2 * d
    PW = W + 2 * d
    in_maps = []
    for m in range(N_CORES):
        r0 = m * ROWS
        in_maps.append(
            {
                "xs": np.ascontiguousarray(
                    px[:, :, r0 : r0 + PR, :].reshape(B * C, PR, PW)
                ),
                "ys": np.ascontiguousarray(
                    py[:, :, r0 : r0 + PR, :].reshape(B * C, PR, PW)
                ),
            }
        )

    res = run_bass_kernel_spmd(nc, in_maps, core_ids=list(range(N_CORES)))

    agg_x = np.concatenate(
        [r["ox"].reshape(B, F * F * C, ROWS, W) for r in res.results], axis=2
    )
    agg_y = np.concatenate(
        [r["oy"].reshape(B, F * F * C, ROWS, W) for r in res.results], axis=2
    )
    return agg_x, agg_y
